# revision 31
# baseline (speedup 1.0000x reference)
"""Trainium2 Bass kernel for nn_CombinatorialClassifierSplit.

Reference computation:
    xr = x.reshape(B, P, S)
    logits = einsum('bps,pks', xr, W) + b          # (B, P, K)
    logp = log_softmax(logits, axis=2)
    out[b, c] = sum_p logp[b, p, idx[p, c]]        # (B, C)

Key restructuring: since idx doesn't depend on b,
    out[b, c] = sum_p logits[b, p, idx[p, c]] - LSE[b]
with LSE[b] = sum_p logsumexp_k(logits[b, p, :]).  The first term is a
plain matmul  M = x_flat @ Wg  where Wg[(p,s), c] = W[p, idx[p,c], s] is a
host-side gather of the *static* index tensor, plus a host-side rank-1
bsum[c] = sum_p b[p, idx[p,c]] correction.  Classes are sharded 8 ways
(CS = 1250/core, split as one 98-wide block + nine 128-wide blocks, no
padding anywhere).

Per core the device computes:
  - the LSE partials for ONLY its 4 partitionings (the p-dimension of the
    softmax stats is data-parallel across cores, killing the 8x replicated
    logits work):  x@W -> +bias -> exp (ACT) -> row-sums (DVE) -> `sums`
    output; the host finishes LSE[b] = sum over all cores' ln(sums).
  - the big matmul (contract 2048) in fp8 DoubleRowSwInterleave mode,
    streamed tile-by-tile (wg is the dominant 2.56MB DMA), with the class
    tiles ordered big->small so the dependent tail (last wg chunk -> +900ns
    DMA sem -> last 2 matmuls -> cast -> out DMA) hangs off a single
    128-class block.
  - psum->sbuf bf16 casts alternate DVE/ACT; outputs ride three HWDGE DMAs
    whose descriptor generations are spread across SP/ACT sequencers so the
    shared HWDGE unit never serializes into the critical tail.
  - zero-operand PE filler matmuls pad every DMA-wait gap so the tensor
    engine's p-state stays ramped (27ns vs 53ns per DoubleRow in the tail).

All matmul operands are fp8e4 (e4m3): x is pre-scaled by 1/2 and W by 2
on the host (the scales cancel in x@W), which centers both operand
distributions inside e4m3's normal range.  M ~ N(0, 5.7) so bf16 output
rounding is ~0.03 versus an error budget of ~3.8.  The bias gather bsum
and the -LSE shift are applied on the host in fp32.
"""

import numpy as np
import ml_dtypes

import concourse.bacc as bacc
import concourse.tile as tile
from concourse import mybir
from concourse.bass_utils import run_bass_kernel_spmd

F8 = ml_dtypes.float8_e4m3
BF16 = ml_dtypes.bfloat16

B, P, K, S, C = 128, 32, 100, 64, 10000
N_CORES = 8
CS = C // N_CORES          # 1250 classes per core
NT = (P * S) // 128        # 16 contract chunks of 128
NPAIR = NT // 2            # DoubleRow processes chunk pairs
PL = P // N_CORES          # 4 local partitionings for the LSE path
TL = PL // 2               # 2 local contract chunks for the LSE path
XSCALE = 0.5               # host: x *= XSCALE, W *= 1/XSCALE (cancels)

# class blocks: block 0 = the core's first 98 classes (plain-matmul tile,
# unpadded); blocks 1-9 = 128-wide DR tiles
BLK_W = [128] * 10

# class tiles: (name, [block indices], [(pair_lo, pair_hi) DMA splits])
# Stream order == list order; the LAST tile is a single 128-class block and
# its final pair ships alone so only 2 DoubleRows + one cast trail the
# +900ns semaphore of the last wg byte.
TILES = [
    ("t0", [0],          [(0, 8)]),
    ("a",  [1, 2, 3, 4], [(0, 2), (2, 4), (4, 6), (6, 8)]),
    ("b",  [5, 6],       [(0, 4), (4, 8)]),
    ("c",  [7],          [(0, 8)]),
    ("d",  [8],          [(0, 8)]),
    ("z",  [9],          [(0, 6), (6, 8)]),
]

# aux tensor layout (fp8): [bias (PL*K) | ones (128)]
AUX_BIAS, AUX_ONES = 0, PL * K
AUX_LEN = PL * K + 128
# xtwk: pure x^T, 16 chunks of 128 cols
XTWK_NC = NT
XTWK_LEN = XTWK_NC * 128

# t0 segment = plain 98-wide gathered block (chunk-major) + the unpadded
# wk shard (2 local chunks x K) appended flat
T0_LEN = NT * 98 + TL * K
WG_LEN = T0_LEN + NPAIR * 2 * 9 * 128   # 20200 fp8 bytes per partition

_cached = {}


def _build_program():
    if "nc" in _cached:
        return _cached["nc"]

    nc = bacc.Bacc("TRN2", target_bir_lowering=False, debug=False,
                   num_devices=N_CORES)
    dt = mybir.dt
    DRI = mybir.MatmulPerfMode.DoubleRowSwInterleave

    xtwk_d = nc.dram_tensor("xtwk", [128, XTWK_NC, 128], dt.float8e4,
                            kind="ExternalInput")
    wg_d = nc.dram_tensor("wg", [128, WG_LEN], dt.float8e4,
                          kind="ExternalInput")
    aux_d = nc.dram_tensor("aux", [1, AUX_LEN], dt.float8e4,
                           kind="ExternalInput")
    # [class-in-block, block, batch] outputs; host transposes back
    # 8th block slot carries the (b,p_local) exp-sums in bf16
    outab_d = nc.dram_tensor("outab", [128, 8, 128], dt.bfloat16,
                             kind="ExternalOutput")
    outc_d = nc.dram_tensor("outc", [128, 128], dt.bfloat16,
                            kind="ExternalOutput")
    outdz_d = nc.dram_tensor("outdz", [128, 2, 128], dt.bfloat16,
                             kind="ExternalOutput")

    with tile.TileContext(nc) as tc:
        with (
            tc.tile_pool(name="const", bufs=1) as cpool,
            tc.tile_pool(name="psum", bufs=8, space="PSUM") as ppool,
        ):
            xtwk_sb = cpool.tile([128, XTWK_NC, 128], dt.float8e4)
            aux_sb = cpool.tile([1, AUX_LEN], dt.float8e4)
            wg_ts = {}
            for name, blks, _sp in TILES:
                if name == "t0":
                    # flat plain-matmul layout: [j, chunk*98 | wk (TL*K)]
                    wg_ts[name] = cpool.tile([128, T0_LEN], dt.float8e4,
                                             name=f"wg_{name}")
                    continue
                nb, w = len(blks), BLK_W[blks[0]]
                wg_ts[name] = cpool.tile([128, NPAIR, nb, 2, w], dt.float8e4,
                                         name=f"wg_{name}")
            exp_sb = cpool.tile([128, PL, K], dt.bfloat16)
            sums_sb = cpool.tile([128, PL], dt.float32)
            zscr_sb = cpool.tile([1, 640], dt.float8e4)
            otab = cpool.tile([128, 8, 128], dt.bfloat16)
            otc = cpool.tile([128, 128], dt.bfloat16)
            otdz = cpool.tile([128, 2, 128], dt.bfloat16)

            bias = lambda lo, n: aux_sb[:, AUX_BIAS + lo:AUX_BIAS + lo + n]
            ones_ap = aux_sb[:, AUX_ONES:AUX_ONES + 128]

            # preload the activation table set holding Exp so the
            # auto-inserted per-function load (1283ns) is skipped
            nc.scalar.add_instruction(mybir.InstLoadActFuncSet(
                name=nc.get_next_instruction_name(), ins=[], outs=[],
                act_func_set_id=6))

            # --- input DMAs, all on SP/HWDGE in exact stream order; the tiny
            # aux rides Pool/SWDGE and slots into a transfer gap ---
            nc.sync.dma_start(xtwk_sb[:], xtwk_d[:])
            nc.gpsimd.dma_start(aux_sb[:], aux_d[:])
            off = 0
            for name, blks, splits in TILES:
                if name == "t0":
                    nc.sync.dma_start(wg_ts[name][:], wg_d[:, off:off + T0_LEN])
                    off += T0_LEN
                    continue
                nb, w = len(blks), BLK_W[blks[0]]
                per_pair = nb * 2 * w
                for (p0, p1) in splits:
                    nc.sync.dma_start(
                        wg_ts[name][:, p0:p1, :, :, :],
                        wg_d[:, off + p0 * per_pair: off + p1 * per_pair]
                        .rearrange("p (a b c d) -> p a b c d",
                                   a=p1 - p0, b=nb, c=2, d=w))
                off += NPAIR * per_pair

            # --- PE warm-up: zero-input matmuls ramp the tensor engine's
            # p-state while the first DMAs are in flight ---
            nc.vector.memset(zscr_sb[:], 0.0)
            fill_ps = ppool.tile([128, 128], dt.float32, tag="ps")

            def fillers(n):
                for _ in range(n):
                    nc.tensor.matmul(fill_ps[:], zscr_sb[:, 0:128],
                                     zscr_sb[:, 128:256],
                                     start=True, stop=True,
                                     skip_group_check=True)

            fillers(4)

            # --- LSE partials for the core's own 4 partitionings:
            # logits -> exp (ACT) -> row sums (DVE) -> tiny f32 output.
            # ln + cross-core sum happen on the HOST. ---
            # The host permutes the 16 contract chunks per core so the core's
            # own 2 logits chunks sit at positions 0-1 (the contract sum of
            # the main matmul is order-agnostic; wg rows are permuted to
            # match).  The SPMD program can then use fixed chunk slices.
            psL = ppool.tile([128, PL * K], dt.float32, tag="ps")
            for tt in range(TL):
                for h in range(2):
                    pl = 2 * tt + h
                    reg = psL[:, pl * K:(pl + 1) * K]
                    nc.tensor.matmul(
                        reg,
                        xtwk_sb[h * 64:h * 64 + 64, tt, :],
                        wg_ts["t0"][h * 64:h * 64 + 64,
                                    NT * 98 + tt * K:NT * 98 + (tt + 1) * K],
                        start=True, stop=False)
                    nc.tensor.matmul(reg, ones_ap, bias(pl * K, K),
                                     start=False, stop=True)
            nc.scalar.activation(exp_sb[:], psL[:, 0:PL * K],
                                 mybir.ActivationFunctionType.Exp)

            fillers(24)

            # --- main fp8 dual-row matmul, tile by tile. Per psum bank the
            # first matmul carries start=True (zeroes the bank); the last DR
            # carries stop. The host applies bsum + (-LSE) afterwards. ---
            ps_t = {}
            for name, blks, _sp in TILES:
                nb, w = len(blks), BLK_W[blks[0]]
                ps_t[name] = ppool.tile([128, nb * 128], dt.float32, tag="ps",
                                        name=f"ps_{name}")

            def tile_drs(name, blks, splits):
                wt, ps = wg_ts[name], ps_t[name]
                if name == "t0":
                    # plain fp8 matmuls, flipped: stationary = x chunk
                    # (128 cols, Ldweights-legal), moving = 98-wide block;
                    # psum lands [batch, class]
                    for t in range(NT):
                        nc.tensor.matmul(
                            ps[:, 0:98],
                            xtwk_sb[:, t, :],
                            wt[:, t * 98:(t + 1) * 98],
                            start=(t == 0), stop=(t == NT - 1),
                            skip_group_check=True)
                    return
                nb, w = len(blks), BLK_W[blks[0]]
                for si, (p0, p1) in enumerate(splits):
                    for pi in range(p0, p1):
                        for bi in range(nb):
                            nc.tensor.matmul(
                                ps[0:w, bi * 128:(bi + 1) * 128],
                                wt[:, pi, bi, :, :],
                                xtwk_sb[:, 2 * pi:2 * pi + 2, :],
                                start=(pi == p0 == 0 and bi == 0),
                                stop=(pi == NPAIR - 1 and bi == nb - 1),
                                perf_mode=DRI, skip_group_check=True)

            # t0 tile
            tile_drs("t0", *_tile("t0"))
            fillers(16)
            tile_drs("a", *_tile("a"))
            fillers(16)
            nc.vector.tensor_scalar_add(otab[:, 0, 0:98], ps_t["t0"][:, 0:98],
                                        0.0)
            fillers(8)
            tile_drs("b", *_tile("b"))
            nc.vector.tensor_scalar_add(otab[:, 1:5, :],
                                        ps_t["a"][:, 0:512], 0.0)
            # reduce + sums output AFTER castA in DVE/Pool program order so
            # the sums transfer's DMA-engine acquire lands behind every input
            # acquire (otherwise its 56ns slips into the middle of the input
            # stream and delays the last wg byte)
            nc.vector.tensor_reduce(sums_sb[:], exp_sb[:],
                                    axis=mybir.AxisListType.X,
                                    op=mybir.AluOpType.add)
            nc.vector.tensor_scalar_add(otab[:, 7, 0:PL], sums_sb[:], 0.0)
            fillers(8)
            tile_drs("c", *_tile("c"))
            nc.scalar.dma_start(outab_d[:, 0:4, :], otab[:, 0:4, :])
            nc.scalar.activation(otab[:, 5:7, :], ps_t["b"][:, 0:256],
                                 mybir.ActivationFunctionType.Copy)
            nc.scalar.dma_start(outab_d[:, 4:8, :], otab[:, 4:8, :])
            fillers(6)
            nc.vector.tensor_scalar_add(otc[:], ps_t["c"][:, 0:128], 0.0)
            nc.sync.dma_start(outc_d[:], otc[:])
            tile_drs("d", *_tile("d"))
            fillers(6)
            nc.scalar.activation(otdz[:, 0, :], ps_t["d"][:, 0:128],
                                 mybir.ActivationFunctionType.Copy)
            tile_drs("z", *_tile("z"))
            nc.vector.tensor_scalar_add(otdz[:, 1, :], ps_t["z"][:, 0:128],
                                        0.0)
            nc.sync.dma_start(outdz_d[:], otdz[:])

    nc.compile()
    _cached["nc"] = nc
    return nc


def _tile(name):
    for n, blks, splits in TILES:
        if n == name:
            return blks, splits
    raise KeyError(name)


def _prep_inputs(x, W, b, idx):
    """Host-side data prep -> per-core input maps."""
    x = np.asarray(x, dtype=np.float32) * XSCALE
    W = np.asarray(W, dtype=np.float32) * (1.0 / XSCALE)
    b = np.asarray(b, dtype=np.float32)
    idx = np.asarray(idx, dtype=np.int64)

    # x^T in (s_local, chunk, b) layout: (128, NT, 128)
    xt = np.ascontiguousarray(
        x.T.reshape(NT, 128, B).transpose(1, 0, 2))

    # gathered big weight matrix: Wg[(p,s), c] = W[p, idx[p,c], s],
    # chunk-major rows: (NT, 128, C)
    Wg = W[np.arange(P)[:, None], idx]            # (P, C, S)
    Wg = np.ascontiguousarray(Wg.transpose(0, 2, 1)).reshape(NT, 128, C)
    bsum_full = b[np.arange(P)[:, None], idx].sum(axis=0)   # (C,)

    aux_base = np.zeros((1, AUX_LEN), dtype=np.float32)
    aux_base[0, AUX_ONES:AUX_ONES + 128] = 1.0

    in_maps = []
    for m in range(N_CORES):
        # per-core chunk permutation: own chunks (2m, 2m+1) first, so the
        # SPMD logits path can address them at fixed positions 0-1
        perm = [2 * m, 2 * m + 1] + [t for t in range(NT)
                                     if t not in (2 * m, 2 * m + 1)]
        xtp = xt[:, perm, :].reshape(128, NT * 128)
        Wgp = Wg[perm].reshape(P * S, C)

        # per-core wk shard: local chunks tt=0,1 are global chunks 2m+tt
        wk = np.empty((128, TL, K), dtype=np.float32)
        for tt in range(TL):
            t = 2 * m + tt
            wk[0:64, tt, :] = W[2 * t].T
            wk[64:128, tt, :] = W[2 * t + 1].T
        xtwk = xtp.reshape(128, NT, 128).astype(F8)

        # per-tile wg shard: t0 = plain chunk-major (98 real classes, no
        # pad); the rest = dual-row interleaved 128-wide blocks
        Wcore = Wgp[:, m * CS:(m + 1) * CS]           # (2048, 1250)
        segs = []
        for name, blks, _sp in TILES:
            if name == "t0":
                t0 = Wcore[:, 0:98].reshape(NT, 128, 98).transpose(1, 0, 2)
                segs.append(np.ascontiguousarray(t0).reshape(128, -1))
                segs.append(wk.reshape(128, TL * K))
                continue
            nb, w = len(blks), BLK_W[blks[0]]
            c_lo = 98 + (blks[0] - 1) * 128
            Wblk = Wcore[:, c_lo:c_lo + nb * w]
            M4 = Wblk.reshape(NPAIR, 2, 128, nb, w)   # [pi, q, j, bi, cc]
            rev = M4[:, :, :, :, ::-1]
            inter = rev.transpose(2, 0, 3, 4, 1)      # [j, pi, bi, cc_r, q]
            segs.append(np.ascontiguousarray(inter).reshape(128, -1))
        wg = np.concatenate(segs, axis=1).astype(F8)
        assert wg.shape[1] == WG_LEN, wg.shape

        aux = aux_base.copy()
        aux[0, AUX_BIAS:AUX_BIAS + PL * K] = \
            b[PL * m:PL * (m + 1)].reshape(-1)
        in_maps.append({"xtwk": xtwk, "wg": wg, "aux": aux.astype(F8),
                        "_bsum": bsum_full[m * CS:(m + 1) * CS]})
    return in_maps


def kernel(x, W, b, partitionings):
    nc = _build_program()
    in_maps = _prep_inputs(x, W, b, partitionings)
    dev_maps = [{k: v for k, v in im.items() if not k.startswith("_")}
                for im in in_maps]
    res = run_bass_kernel_spmd(nc, dev_maps, list(range(N_CORES)))

    # LSE[b] = sum over all 32 p of ln(exp-sum); each core did 4 p's,
    # shipped bf16 in otab slot 7
    sums = np.concatenate(
        [np.asarray(res.results[m]["outab"])[:, 7, 0:PL].astype(np.float32)
         for m in range(N_CORES)], axis=1)                    # (128, 32)
    lse = np.log(sums).sum(axis=1, keepdims=True)             # (128, 1)

    cores = []
    for m in range(N_CORES):
        r = res.results[m]
        blkcols = []
        ab = np.asarray(r["outab"]).astype(np.float32)        # (128, 7, 128)
        oc = np.asarray(r["outc"]).astype(np.float32)         # (128, 128)
        dz = np.asarray(r["outdz"]).astype(np.float32)        # (128, 2, 128)
        blkcols.append(ab[:, 0, 0:98])                        # block 0 (98)
        for k in range(1, 7):
            blkcols.append(ab[:, k, :].T)                     # blocks 1-6
        blkcols.append(oc.T)                                  # block 7
        blkcols.append(dz[:, 0, :].T)                         # block 8
        blkcols.append(dz[:, 1, :].T)                         # block 9
        core_out = np.concatenate(blkcols, axis=1)            # (128, 1250)
        core_out += in_maps[m]["_bsum"][None, :]
        cores.append(core_out)
    out = np.concatenate(cores, axis=1)
    return (out - lse).astype(np.float32)
